# revision 1
# baseline (speedup 1.0000x reference)
"""Trainium2 Bass kernel for nn_BoxHead (NMS detection head).

Data-parallel over the batch: each of the 8 NeuronCores processes one image
end-to-end (log-softmax scoring, top-candidate selection via per-partition
top-8 + matmul-based ranking/scatter, sparse indirect-DMA gather of the
needed box-regression/proposal rows, box decode, IoU suppression matrix,
Jacobi-iterated greedy NMS, and top-100 output assembly).

Self-contained: builds the Bass program on import-path /opt/trn_rl_repo and
runs it on cores 0-7 via run_bass_kernel_spmd.
"""
import sys
sys.path.insert(0, "/opt/trn_rl_repo")

import numpy as np
import concourse.bacc as bacc
import concourse.mybir as mybir
from concourse import tile, bass

dt = mybir.dt
F32 = dt.float32
AX = mybir.AxisListType
OP = mybir.AluOpType
ACT = mybir.ActivationFunctionType

N, C = 4000, 91
NT = 32
KW = 90 * NT
LW = 91 * NT
LOG05 = float(np.log(0.05))
CLIP = float(np.log(1000.0 / 16.0))
NEG = -60000.0
IMG = 800.0

C_IOTA256 = 0
C_TRI = 256
C_ID = 384
C_ONES = 512
C_IOTAP0 = 640
C_IOTAP1 = 641
C_NEG = 642
C_127 = 643
C_BND = 644
C_W = 704


def make_consts():
    cst = np.zeros((128, C_W), np.float32)
    cst[:, C_IOTA256:C_IOTA256 + 256] = np.arange(256, dtype=np.float32)[None, :]
    p = np.arange(128)
    cst[:, C_TRI:C_TRI + 128] = (p[:, None] < p[None, :]).astype(np.float32)
    cst[:, C_ID:C_ID + 128] = np.eye(128, dtype=np.float32)
    cst[:, C_ONES:C_ONES + 128] = 1.0
    cst[:, C_IOTAP0] = p
    cst[:, C_IOTAP1] = p + 128.0
    cst[:, C_NEG] = NEG
    cst[:, C_127] = 127.0
    cst[:, C_BND:C_BND + 31] = 90.0 * np.arange(1, 32, dtype=np.float32)[None, :]
    return cst


def prep_logits(logits_img):
    lg = np.zeros((128, LW), np.float32)
    lg[:, 91 * 31] = 40.0
    for t in range(NT):
        rows = logits_img[128 * t:128 * (t + 1)]
        lg[:rows.shape[0], 91 * t:91 * t + 91] = rows
    return lg


def prep_rg8(reg_img, props_img):
    out = np.empty((N, 91, 8), np.float32)
    out[:, :, 0:4] = reg_img.reshape(N, 91, 4)
    out[:, :, 4:8] = props_img[:, None, :]
    return out.reshape(N * 91, 8)


def build():
    nc = bacc.Bacc("TRN2", target_bir_lowering=False, debug=False, num_devices=8)
    lg_d = nc.dram_tensor("lg", [128, LW], F32, kind="ExternalInput")
    rg_d = nc.dram_tensor("rg", [N * 91, 8], F32, kind="ExternalInput")
    cst_d = nc.dram_tensor("cst", [128, C_W], F32, kind="ExternalInput")
    ob_d = nc.dram_tensor("ob", [100, 4], F32, kind="ExternalOutput")
    os_d = nc.dram_tensor("os", [100, 1], F32, kind="ExternalOutput")
    ol_d = nc.dram_tensor("ol", [100, 1], dt.int32, kind="ExternalOutput")
    with tile.TileContext(nc) as tc:
        _body(nc, tc, lg_d, rg_d, cst_d, ob_d, os_d, ol_d)
    nc.compile()
    return nc


def _body(nc, tc, lg_d, rg_d, cst_d, ob_d, os_d, ol_d):
    with tc.tile_pool(name="main", bufs=1) as P, \
         tc.tile_pool(name="ps", bufs=1, space="PSUM") as PS:

        cst = P.tile([128, C_W], F32, tag="cst", name="cst")
        nc.sync.dma_start(out=cst[:], in_=cst_d[:])
        dummy = P.tile([1, 2], F32, tag="dummy", name="dummy")
        nc.scalar.activation(dummy[:, 0:1], cst[0:1, C_127:C_127 + 1], ACT.Exp)
        lg = P.tile([128, LW], F32, tag="lg", name="lg")
        HLW = LW // 2
        nc.sync.dma_start(out=lg[:, 0:HLW], in_=lg_d[:, 0:HLW])
        nc.sync.dma_start(out=lg[:, HLW:LW], in_=lg_d[:, HLW:LW])

        iota128 = cst[:, C_IOTA256:C_IOTA256 + 128]
        tri = cst[:, C_TRI:C_TRI + 128]
        ident = cst[:, C_ID:C_ID + 128]
        ones = cst[:, C_ONES:C_ONES + 128]
        iotaP = cst[:, C_IOTAP0:C_IOTAP0 + 1]
        negc = cst[:, C_NEG:C_NEG + 1]
        bnd = cst[:, C_BND:C_BND + 31]

        # ---- 1. log-softmax keys ----
        key = P.tile([128, KW], F32, tag="key", name="key")
        with nc.named_scope("softmax"):
            et = P.tile([128, LW], F32, tag="et", name="et")
            zs = P.tile([128, NT], F32, tag="zs", name="zs")
            ngr = P.tile([128, NT], F32, tag="ngr", name="ngr")
            nc.scalar.activation(et[:, 0:HLW], lg[:, 0:HLW], ACT.Exp)
            nc.vector.tensor_reduce(zs[:, 0:16], et[:, 0:HLW].rearrange("p (t c) -> p t c", c=91),
                                    axis=AX.X, op=OP.add)
            nc.scalar.activation(et[:, HLW:LW], lg[:, HLW:LW], ACT.Exp)
            nc.vector.tensor_reduce(zs[:, 16:32], et[:, HLW:LW].rearrange("p (t c) -> p t c", c=91),
                                    axis=AX.X, op=OP.add)
            nc.scalar.activation(ngr[:], zs[:], ACT.Ln)
            nc.scalar.activation(dummy[:, 1:2], cst[0:1, C_127:C_127 + 1], ACT.Exp)
            nc.vector.tensor_scalar(out=ngr[:], in0=ngr[:], scalar1=-1.0, scalar2=None, op0=OP.mult)
            nc.vector.tensor_tensor(
                out=key[:].rearrange("p (t c) -> p t c", c=90),
                in0=lg[:].rearrange("p (t c) -> p t c", c=91)[:, :, 1:91],
                in1=ngr[:].rearrange("p (t o) -> p t o", o=1).to_broadcast([128, NT, 90]),
                op=OP.add)

        # ---- 2. per-partition top-8 ----
        max8 = P.tile([128, 8], F32, tag="max8", name="max8")
        idx8u = P.tile([128, 8], dt.uint32, tag="idx8u", name="idx8u")
        idx8 = P.tile([128, 8], F32, tag="idx8", name="idx8")
        with nc.named_scope("top8"):
            nc.vector.max(out=max8[:], in_=key[:])
            nc.vector.max_index(out=idx8u[:], in_max=max8[:], in_values=key[:])
            nc.vector.tensor_copy(out=idx8[:], in_=idx8u[:])

        # ---- 3. pool rank ----
        rank = P.tile([128, 8], F32, tag="rank", name="rank")
        with nc.named_scope("rank1"):
            tp8 = PS.tile([8, 128], F32, tag="psA", name="tp8", space="PSUM")
            nc.tensor.transpose(out=tp8[:], in_=max8[:], identity=ident)
            tp8s = P.tile([8, 128], F32, tag="tp8s", name="tp8s")
            nc.scalar.copy(out=tp8s[:], in_=tp8[:])
            poolflat = P.tile([1, 1024], F32, tag="poolflat", name="poolflat")
            nc.sync.dma_start(out=poolflat[:], in_=tp8s[:])
            rep = PS.tile([128, 1024], F32, tag="psB", name="rep", space="PSUM")
            nc.tensor.matmul(out=rep[:, 0:512], lhsT=ones[0:1, :], rhs=poolflat[:, 0:512], start=True, stop=True)
            nc.tensor.matmul(out=rep[:, 512:1024], lhsT=ones[0:1, :], rhs=poolflat[:, 512:1024], start=True, stop=True)
            scrap = P.tile([128, 1024], F32, tag="scrap", name="scrap")
            for j in range(8):
                nc.vector.tensor_scalar(out=scrap[:], in0=rep[:], scalar1=max8[:, j:j + 1],
                                        scalar2=0.0, op0=OP.is_gt, op1=OP.add,
                                        accum_out=rank[:, j:j + 1])

        # ---- 4. scatter to dense top-128 (key, lidx, p) ----
        fields = P.tile([128, 24], F32, tag="fieldsA", name="fieldsA")
        f3 = fields[:].rearrange("p (j f) -> p j f", f=3)
        nc.vector.tensor_copy(out=f3[:, :, 0], in_=max8[:])
        nc.vector.tensor_copy(out=f3[:, :, 1], in_=idx8[:])
        nc.vector.tensor_copy(out=f3[:, :, 2], in_=iotaP.to_broadcast([128, 8]))
        dense = P.tile([128, 3], F32, tag="dense", name="dense")
        with nc.named_scope("scatterA"):
            gj = P.tile([128, 128], F32, tag="gj", name="gj")
            dps = PS.tile([128, 3], F32, tag="psC", name="dps", space="PSUM")
            for j in range(8):
                nc.vector.tensor_scalar(out=gj[:], in0=iota128, scalar1=rank[:, j:j + 1],
                                        scalar2=None, op0=OP.is_equal)
                nc.tensor.matmul(out=dps[:], lhsT=gj[:], rhs=fields[:, 3 * j:3 * j + 3],
                                 start=(j == 0), stop=(j == 7))
            nc.vector.tensor_copy(out=dense[:], in_=dps[:])

        # ---- 5. offsets + gather + decode ----
        ff = P.tile([128, 8], F32, tag="ff", name="ff")    # x1 y1 x2 y2 key label area .
        g8 = P.tile([128, 8], F32, tag="g8", name="g8")    # dx dy dw dh px1 py1 px2 py2
        wk = P.tile([128, 8], F32, tag="wk", name="wk")
        vmi = P.tile([128, 1], dt.uint8, tag="vmi", name="vmi")
        scr31 = P.tile([128, 31], F32, tag="scr31", name="scr31")
        oreg_i = P.tile([128, 1], dt.int32, tag="oreg_i", name="oreg_i")
        dkey = dense[:, 0:1]
        dlidx = dense[:, 1:2]
        dp = dense[:, 2:3]
        with nc.named_scope("gather"):
            tcol, oreg_f = wk[:, 0:1], wk[:, 1:2]
            nc.vector.tensor_scalar(out=scr31[:], in0=bnd, scalar1=dlidx,
                                    scalar2=0.0, op0=OP.is_le, op1=OP.add, accum_out=tcol)
            nc.vector.scalar_tensor_tensor(out=oreg_f, in0=tcol, scalar=11558.0, in1=dlidx,
                                           op0=OP.mult, op1=OP.add)
            nc.vector.scalar_tensor_tensor(out=oreg_f, in0=dp, scalar=91.0, in1=oreg_f,
                                           op0=OP.mult, op1=OP.add)
            nc.vector.tensor_scalar(out=oreg_f, in0=oreg_f, scalar1=1.0, scalar2=None, op0=OP.add)
            nc.vector.scalar_tensor_tensor(out=ff[:, 5:6], in0=tcol, scalar=-90.0, in1=dlidx,
                                           op0=OP.mult, op1=OP.add)
            nc.vector.tensor_copy(out=oreg_i[:], in_=oreg_f)
            nc.gpsimd.indirect_dma_start(out=g8[:], out_offset=None, in_=rg_d[:],
                                         in_offset=bass.IndirectOffsetOnAxis(ap=oreg_i[:, 0:1], axis=0))
        with nc.named_scope("decode"):
            wbox, hbox, cx, cy, pcx, pcy = (wk[:, i:i + 1] for i in range(2, 8))
            nc.vector.tensor_tensor(out=wbox, in0=g8[:, 6:7], in1=g8[:, 4:5], op=OP.subtract)
            nc.vector.tensor_tensor(out=hbox, in0=g8[:, 7:8], in1=g8[:, 5:6], op=OP.subtract)
            nc.vector.scalar_tensor_tensor(out=cx, in0=wbox, scalar=0.5, in1=g8[:, 4:5],
                                           op0=OP.mult, op1=OP.add)
            nc.vector.scalar_tensor_tensor(out=cy, in0=hbox, scalar=0.5, in1=g8[:, 5:6],
                                           op0=OP.mult, op1=OP.add)
            nc.vector.tensor_scalar(out=g8[:, 2:4], in0=g8[:, 2:4], scalar1=0.2, scalar2=CLIP,
                                    op0=OP.mult, op1=OP.min)
            nc.scalar.activation(g8[:, 2:4], g8[:, 2:4], ACT.Exp)
            pw_, ph_ = g8[:, 2:3], g8[:, 3:4]
            nc.vector.tensor_tensor(out=pw_, in0=pw_, in1=wbox, op=OP.mult)
            nc.vector.tensor_tensor(out=ph_, in0=ph_, in1=hbox, op=OP.mult)
            nc.vector.tensor_scalar(out=g8[:, 0:2], in0=g8[:, 0:2], scalar1=0.1,
                                    scalar2=None, op0=OP.mult)
            nc.vector.tensor_tensor(out=pcx, in0=g8[:, 0:1], in1=wbox, op=OP.mult)
            nc.vector.tensor_tensor(out=pcx, in0=pcx, in1=cx, op=OP.add)
            nc.vector.tensor_tensor(out=pcy, in0=g8[:, 1:2], in1=hbox, op=OP.mult)
            nc.vector.tensor_tensor(out=pcy, in0=pcy, in1=cy, op=OP.add)
            nc.vector.scalar_tensor_tensor(out=ff[:, 0:1], in0=pw_, scalar=-0.5, in1=pcx,
                                           op0=OP.mult, op1=OP.add)
            nc.vector.scalar_tensor_tensor(out=ff[:, 2:3], in0=pw_, scalar=0.5, in1=pcx,
                                           op0=OP.mult, op1=OP.add)
            nc.vector.scalar_tensor_tensor(out=ff[:, 1:2], in0=ph_, scalar=-0.5, in1=pcy,
                                           op0=OP.mult, op1=OP.add)
            nc.vector.scalar_tensor_tensor(out=ff[:, 3:4], in0=ph_, scalar=0.5, in1=pcy,
                                           op0=OP.mult, op1=OP.add)
            nc.vector.tensor_scalar(out=ff[:, 0:4], in0=ff[:, 0:4], scalar1=0.0, scalar2=IMG,
                                    op0=OP.max, op1=OP.min)
            gw, gh, vm, gk = wk[:, 2:3], wk[:, 3:4], wk[:, 4:5], wk[:, 5:6]
            nc.vector.tensor_tensor(out=gw, in0=ff[:, 2:3], in1=ff[:, 0:1], op=OP.subtract)
            nc.vector.tensor_tensor(out=gh, in0=ff[:, 3:4], in1=ff[:, 1:2], op=OP.subtract)
            nc.vector.tensor_scalar(out=gw, in0=gw, scalar1=1.0, scalar2=None, op0=OP.is_ge)
            nc.vector.tensor_scalar(out=gh, in0=gh, scalar1=1.0, scalar2=None, op0=OP.is_ge)
            nc.vector.tensor_tensor(out=vm, in0=gw, in1=gh, op=OP.mult)
            nc.vector.tensor_scalar(out=gk, in0=dkey, scalar1=LOG05, scalar2=None, op0=OP.is_gt)
            nc.vector.tensor_tensor(out=vm, in0=vm, in1=gk, op=OP.mult)
            nc.vector.tensor_scalar(out=gk, in0=dkey, scalar1=-0.01, scalar2=None, op0=OP.is_lt)
            nc.vector.tensor_tensor(out=vm, in0=vm, in1=gk, op=OP.mult)
            nc.vector.tensor_copy(out=vmi[:], in_=vm)
            nc.vector.tensor_copy(out=ff[:, 4:5], in_=negc)
            nc.vector.copy_predicated(out=ff[:, 4:5], mask=vmi[:], data=dkey)
            # area
            nc.vector.tensor_tensor(out=gw, in0=ff[:, 2:3], in1=ff[:, 0:1], op=OP.subtract)
            nc.vector.tensor_tensor(out=gh, in0=ff[:, 3:4], in1=ff[:, 1:2], op=OP.subtract)
            nc.vector.tensor_tensor(out=ff[:, 6:7], in0=gw, in1=gh, op=OP.mult)

        # ---- 6. transpose fields + replicate ----
        with nc.named_scope("rank2"):
            tpf = PS.tile([6, 128], F32, tag="psF", name="tpf", space="PSUM")
            nc.tensor.transpose(out=tpf[:], in_=ff[:, 0:6], identity=ident)
            spsTs = P.tile([6, 128], F32, tag="spsTs", name="spsTs")
            nc.scalar.copy(out=spsTs[:], in_=tpf[:])
            fflat = P.tile([1, 640], F32, tag="fflat", name="fflat")
            nc.sync.dma_start(out=fflat[:, 0:512], in_=spsTs[0:4, :])
            nc.sync.dma_start(out=fflat[:, 512:640], in_=spsTs[5:6, :])
            repf = PS.tile([128, 640], F32, tag="psB", name="repf", space="PSUM")
            nc.tensor.matmul(out=repf[:, 0:512], lhsT=ones[0:1, :], rhs=fflat[:, 0:512], start=True, stop=True)
            nc.tensor.matmul(out=repf[:, 512:640], lhsT=ones[0:1, :], rhs=fflat[:, 512:640], start=True, stop=True)
            R = {nm: repf[:, 128 * fi:128 * (fi + 1)] for fi, nm in
                 enumerate(["x1", "y1", "x2", "y2", "lb"])}
            xy1 = P.tile([128, 256], F32, tag="xy1", name="xy1")
            nc.vector.tensor_copy(out=xy1[:], in_=repf[:, 0:256])
            ar0 = P.tile([128, 128], F32, tag="ar0", name="ar0")
            arR = P.tile([128, 128], F32, tag="arR", name="arR")
            nc.vector.tensor_tensor(out=ar0[:], in0=R["x2"], in1=xy1[:, 0:128], op=OP.subtract)
            nc.vector.tensor_tensor(out=arR[:], in0=R["y2"], in1=xy1[:, 128:256], op=OP.subtract)
            nc.vector.tensor_tensor(out=arR[:], in0=arR[:], in1=ar0[:], op=OP.mult)
            R["ar"] = arR[:]

        # ---- 7. suppression matrix [128, 128] ----
        S0 = P.tile([128, 128], F32, tag="S0", name="S0")
        with nc.named_scope("iou"):
            xx = P.tile([128, 128], F32, tag="xx", name="xx")
            yy = P.tile([128, 128], F32, tag="yy", name="yy")
            nc.vector.tensor_scalar(out=xx[:], in0=R["x2"], scalar1=ff[:, 2:3], scalar2=None, op0=OP.min)
            nc.vector.tensor_scalar(out=S0[:], in0=R["x1"], scalar1=ff[:, 0:1], scalar2=None, op0=OP.max)
            nc.vector.tensor_tensor(out=xx[:], in0=xx[:], in1=S0[:], op=OP.subtract)
            nc.vector.tensor_scalar(out=xx[:], in0=xx[:], scalar1=0.0, scalar2=None, op0=OP.max)
            nc.vector.tensor_scalar(out=yy[:], in0=R["y2"], scalar1=ff[:, 3:4], scalar2=None, op0=OP.min)
            nc.vector.tensor_scalar(out=S0[:], in0=R["y1"], scalar1=ff[:, 1:2], scalar2=None, op0=OP.max)
            nc.vector.tensor_tensor(out=yy[:], in0=yy[:], in1=S0[:], op=OP.subtract)
            nc.vector.scalar_tensor_tensor(out=xx[:], in0=xx[:], scalar=3.0, in1=yy[:],
                                           op0=OP.mult, op1=OP.mult)
            nc.vector.tensor_scalar(out=yy[:], in0=R["ar"], scalar1=ff[:, 6:7], scalar2=None, op0=OP.add)
            nc.vector.tensor_tensor(out=xx[:], in0=xx[:], in1=yy[:], op=OP.is_gt)
            nc.vector.tensor_scalar(out=yy[:], in0=R["lb"], scalar1=ff[:, 5:6], scalar2=None, op0=OP.is_equal)
            nc.vector.tensor_tensor(out=xx[:], in0=xx[:], in1=yy[:], op=OP.mult)
            nc.vector.tensor_scalar(out=yy[:], in0=iota128, scalar1=iotaP, scalar2=None, op0=OP.is_gt)
            nc.vector.tensor_tensor(out=S0[:], in0=xx[:], in1=yy[:], op=OP.mult)

        # ---- 8. Jacobi NMS ----
        NITER = 2
        keep = P.tile([128, 1], F32, tag="kp_0", name="kp_0")
        valid0 = P.tile([128, 1], F32, tag="v0", name="v0")
        with nc.named_scope("nms"):
            nc.vector.tensor_scalar(out=keep[:], in0=ff[:, 4:5], scalar1=-50000.0,
                                    scalar2=None, op0=OP.is_gt)
            nc.vector.tensor_copy(out=valid0[:], in_=keep[:])
            for it in range(NITER):
                nkeep = P.tile([128, 1], F32, tag=f"kp_{it + 1}", name=f"kp_{it + 1}")
                sup = PS.tile([128, 1], F32, tag="psC", name=f"sup{it}", space="PSUM")
                nc.tensor.matmul(out=sup[:], lhsT=S0[:], rhs=keep[:], start=True, stop=True)
                nc.vector.scalar_tensor_tensor(out=nkeep[:], in0=sup[:], scalar=0.5,
                                               in1=valid0[:], op0=OP.is_le, op1=OP.mult)
                keep = nkeep

        # ---- 9. output ----
        with nc.named_scope("out"):
            pos = PS.tile([128, 1], F32, tag="psD", name="pos", space="PSUM")
            nc.tensor.matmul(out=pos[:], lhsT=tri, rhs=keep[:], start=True, stop=True)
            outp = PS.tile([128, 6], F32, tag="psA", name="outp", space="PSUM")
            g3 = P.tile([128, 128], F32, tag="g3", name="g3")
            slot = P.tile([128, 1], F32, tag="slot", name="slot")
            nc.scalar.activation(ff[:, 4:5], ff[:, 4:5], ACT.Exp)
            nc.vector.tensor_scalar(out=ff[:, 5:6], in0=ff[:, 5:6], scalar1=2.0,
                                    scalar2=None, op0=OP.add)
            nc.vector.scalar_tensor_tensor(out=slot[:], in0=pos[:], scalar=-127.0,
                                           in1=keep[:], op0=OP.add, op1=OP.mult)
            nc.vector.tensor_scalar(out=slot[:], in0=slot[:], scalar1=127.0, scalar2=None, op0=OP.add)
            nc.vector.tensor_tensor(out=g3[:], in0=iota128,
                                    in1=slot[:].to_broadcast([128, 128]), op=OP.is_equal)
            nc.tensor.matmul(out=outp[:], lhsT=g3[:], rhs=ff[:, 0:6], start=True, stop=True)
            outs = P.tile([128, 6], F32, tag="outs", name="outs")
            nc.vector.tensor_copy(out=outs[:], in_=outp[:])
            nc.vector.tensor_scalar(out=outs[:, 5:6], in0=outs[:, 5:6], scalar1=1.0, scalar2=None, op0=OP.subtract)
            labi = P.tile([128, 1], dt.int32, tag="labi", name="labi")
            nc.vector.tensor_copy(out=labi[:], in_=outs[:, 5:6])
            nc.sync.dma_start(out=ob_d[:], in_=outs[0:100, 0:4])
            nc.sync.dma_start(out=os_d[:], in_=outs[0:100, 4:5])
            nc.sync.dma_start(out=ol_d[:], in_=labi[0:100, :])


_NC_CACHE = {}


def _get_nc():
    if "nc" not in _NC_CACHE:
        _NC_CACHE["nc"] = build()
    return _NC_CACHE["nc"]


def kernel(class_logits, box_regression, proposals, img_h=800, img_w=800):
    """Full-input entry point: shards per image across 8 NeuronCores."""
    from concourse.bass_utils import run_bass_kernel_spmd
    B = proposals.shape[0]
    assert B == 8 and int(img_h) == 800 and int(img_w) == 800
    logits = np.ascontiguousarray(np.asarray(class_logits, np.float32)).reshape(B, N, C)
    reg = np.ascontiguousarray(np.asarray(box_regression, np.float32)).reshape(B, N, 4 * C)
    props = np.ascontiguousarray(np.asarray(proposals, np.float32))
    cst = make_consts()
    in_maps = [{"lg": prep_logits(logits[b]),
                "rg": prep_rg8(reg[b], props[b]),
                "cst": cst} for b in range(B)]
    nc = _get_nc()
    res = run_bass_kernel_spmd(nc, in_maps, list(range(8)))
    boxes = np.stack([res.results[b]["ob"] for b in range(B)])
    scores = np.stack([res.results[b]["os"][:, 0] for b in range(B)])
    labels = np.stack([res.results[b]["ol"][:, 0] for b in range(B)]).astype(np.int32)
    return boxes, scores, labels
